# revision 3
# baseline (speedup 1.0000x reference)
"""MLA forward on 8 Trainium2 NeuronCores — zero-collective design.

Each core owns one batch (4 cores per batch) and 512 query tokens arranged as
8 slots of 64, chosen so every core's causal attention has identical shape
(slot s attends 2s+2 key-tiles of 128). The kv path (compress + decompress of
all 16 heads for the full 2048 positions) is replicated across the batch's 4
cores, which removes all collectives. Output projection is local to each
core's tokens; the host reassembles.
"""

import numpy as np

import concourse.bacc as bacc
import concourse.bass_isa as bass_isa
import concourse.mybir as mybir
import concourse.tile as tile
from concourse import bass_utils

B, S, D = 2, 2048, 2048
H = 16
NOPE, ROPE, VH = 128, 64, 128
HALF = ROPE // 2
QR = KVR = 512
EPS = 1e-6
W = 8                      # cores
LQ = 512                   # local q tokens per core
NSLOT = 8                  # q slots of 64
QW = 64
NKT = 16                   # key tiles of 128
SCALE = 1.0 / np.sqrt(NOPE + ROPE)
HPP = 8                    # heads per pass
NPASS = 2
# ownership: key tile b owned by core b%4; AG half t carries the owner's
# tile-slots {2t, 2t+1}; natural tile b sits at gathered block GPERM[b].
GPERM = [8 * ((b // 4) // 2) + 2 * (b % 4) + ((b // 4) % 2) for b in range(16)]

FP16 = mybir.dt.float16
FP32 = mybir.dt.float32
FP8 = mybir.dt.float8e4
FP8E5 = mybir.dt.float8e5
DRM = mybir.MatmulPerfMode.DoubleRow
AF = mybir.ActivationFunctionType

DEBUG = False
USE_FP8 = False            # compress/decompress/scores path in fp8-DoubleRow
USE_FP8_PT = False         # attention probabilities in fp8e5 (AV/pz DoubleRow)

_cache = {}


def _build():
    fp8 = USE_FP8
    fp8pt = USE_FP8_PT and fp8
    dtA = FP8 if fp8 else FP16
    dtP = FP8E5 if fp8pt else FP16

    nc = bacc.Bacc("TRN2", target_bir_lowering=False, debug=False)

    def din(name, shape, dt=FP16):
        return nc.dram_tensor(name, shape, dt, kind="ExternalInput").ap()

    x_kv = din("x_kv", [128, 16, 512], dtA)   # this core's kv positions (p%4==j)
    x_q = din("x_q", [128, 16, LQ], dtA)
    w_cq = din("w_cq", [128, 16, QR], dtA)
    w_ckx = din("w_ckx", [128, 16, 640], dtA)      # ckv 512 | krope p2 64 | krope r2 64
    w_dqn = din("w_dqn", [128, 4, H * NOPE], dtA)
    w_dqr2 = din("w_dqr2", [128, 4, H * 128], dtA)  # per head-pair: p2|p2|r2|r2 blocks
    w_dkn = din("w_dkn", [128, 4, H * NOPE], dtA)
    w_dv = din("w_dv", [128, 4, H * VH], dtA)
    w_proj = din("w_proj", [128, 16, D])
    cs_kv = din("cs_kv", [64, 512])
    msc_kv = din("msc_kv", [64, 512])
    cs_q2 = din("cs_q2", [128, LQ])
    msc_q2 = din("msc_q2", [128, LQ])
    masks = din("masks", [128, 16, QW])             # additive 0/-10000 per slot/iter
    out_c = nc.dram_tensor("out_c", [128, 16, LQ], FP32, kind="ExternalOutput").ap()
    if DEBUG:
        dbg_kvlat = nc.dram_tensor("dbg_kvlat", [128, 4, S], dtA, kind="ExternalOutput").ap()
        dbg_qlat = nc.dram_tensor("dbg_qlat", [128, 4, LQ], dtA, kind="ExternalOutput").ap()
        dbg_krope = nc.dram_tensor("dbg_krope", [128, S], dtA, kind="ExternalOutput").ap()
        dbg_kn = nc.dram_tensor("dbg_kn", [128, HPP, S], FP16, kind="ExternalOutput").ap()
        dbg_vt = nc.dram_tensor("dbg_vt", [128, NKT, HPP * VH], FP16, kind="ExternalOutput").ap()
        dbg_qn = nc.dram_tensor("dbg_qn", [128, HPP, LQ], FP16, kind="ExternalOutput").ap()
        dbg_qr2 = nc.dram_tensor("dbg_qr2", [128, HPP // 2, LQ], FP16, kind="ExternalOutput").ap()
        dbg_ao = nc.dram_tensor("dbg_ao", [128, H, LQ], FP16, kind="ExternalOutput").ap()
        dbg_pt = nc.dram_tensor("dbg_pt", [128, 2, 512], FP16, kind="ExternalOutput").ap()
        dbg_knv2 = nc.dram_tensor("dbg_knv2", [128, HPP, 2, S], FP8, kind="ExternalOutput").ap()
        dbg_qdr = nc.dram_tensor("dbg_qdr", [128, 2 * HPP, 2, LQ], FP8, kind="ExternalOutput").ap()
        dbg_vt8 = nc.dram_tensor("dbg_vt8", [128, NKT, HPP * VH],
                                 FP8 if (USE_FP8 and USE_FP8_PT) else FP16,
                                 kind="ExternalOutput").ap()
        dbg_pav = nc.dram_tensor("dbg_pav", [128, 512], FP32, kind="ExternalOutput").ap()
        dbg_pz = nc.dram_tensor("dbg_pz", [128, 512], FP32, kind="ExternalOutput").ap()

    nch = 2 if fp8 else 1   # contraction step: chunks consumed per matmul

    def contract(ps_ap, w_t, wc, wsl, r_t, rc, rsl, n):
        """Accumulate ps += sum over n chunks: w[:, c, wsl].T @ r[:, c, rsl].
        wc/rc index the chunk dim of 3D tiles w_t/r_t."""
        if fp8:
            for i in range(n // 2):
                nc.tensor.matmul(
                    ps_ap,
                    w_t[:, wc + 2 * i:wc + 2 * i + 2, wsl],
                    r_t[:, rc + 2 * i:rc + 2 * i + 2, rsl],
                    start=(i == 0), stop=(i == n // 2 - 1), perf_mode=DRM)
        else:
            for i in range(n):
                nc.tensor.matmul(
                    ps_ap, w_t[:, wc + i, wsl], r_t[:, rc + i, rsl],
                    start=(i == 0), stop=(i == n - 1))

    cp_engines = None

    def cp(dst, src):
        cp_engines.append(cp_engines.pop(0))
        eng = cp_engines[-1]
        if eng is nc.scalar:
            eng.copy(dst, src)
        else:
            eng.tensor_copy(dst, src)

    with tile.TileContext(nc) as tc:
        cp_engines = [nc.vector, nc.scalar]

        const_cm = tc.tile_pool(name="const", bufs=1)
        const = const_cm.__enter__()
        zero1 = const.tile([128, 1], FP32, name="zero1")
        nc.any.memset(zero1[:], 0.0)
        eps_t = const.tile([1, 1], FP32, name="eps_t")
        nc.any.memset(eps_t[:], EPS)
        invn = const.tile([128, 1], FP16, name="invn")
        nc.any.memset(invn[:], 1.0 / QR)
        ones1 = const.tile([1, 128], FP16, name="ones1")
        nc.any.memset(ones1[:], 1.0)
        if fp8pt:
            ones_h = const.tile([128, 2, 128], FP8, name="ones_h")
        else:
            ones_h = const.tile([128, 128], FP16, name="ones_h")
        nc.any.memset(ones_h[:], 1.0)
        cs_q2_t = const.tile([128, LQ], FP16, name="cs_q2_t")
        msc_q2_t = const.tile([128, LQ], FP16, name="msc_q2_t")
        nc.gpsimd.dma_start(cs_q2_t[:], cs_q2[:])
        nc.gpsimd.dma_start(msc_q2_t[:], msc_q2[:])

        # persistent activations
        lat_cm = tc.tile_pool(name="lat", bufs=1)
        lat_pool = lat_cm.__enter__()
        kvlat_h = [lat_pool.tile([128, 4, S // 2], dtA, name=f"kvlat{t}")
                   for t in range(2)]
        qlat = lat_pool.tile([128, 4, LQ], dtA, name="qlat")
        krope_h = [lat_pool.tile([128, S // 2], dtA, name=f"krope{t}")
                   for t in range(2)]

        # ---------------- Phase A: compress own positions + AllGather --------
        # Each core compresses only its own 512 kv positions (p % 4 == j in
        # its batch), then the 4 batch cores AllGather the latents+krope in
        # two pipelined halves. The gathered loads undo the position
        # interleave so decompress sees natural key order.
        dram_cm = tc.tile_pool(name="dram", bufs=1, space="DRAM")
        dram = dram_cm.__enter__()
        lat_kin = [dram.tile([576, 256], dtA, tag=f"lat_kin{t}",
                             name=f"lat_kin{t}") for t in range(2)]
        lat_g = [dram.tile([4, 576, 256], dtA, tag=f"lat_g{t}",
                           name=f"lat_g{t}") for t in range(2)]

        with tc.tile_pool(name="cmp_x", bufs=1) as cmp_x, \
             tc.tile_pool(name="cmp_w", bufs=1) as cmp_w, \
             tc.tile_pool(name="cmp_t", bufs=2) as cmp_t, \
             tc.tile_pool(name="ps_cmp", bufs=1, space="PSUM") as ps_cmp, \
             tc.tile_pool(name="ps_nrm", bufs=1, space="PSUM") as ps_nrm:
            cs_kv_t = cmp_x.tile([64, 512], FP16, name="cs_kv_t")
            msc_kv_t = cmp_x.tile([64, 512], FP16, name="msc_kv_t")
            nc.gpsimd.dma_start(cs_kv_t[:], cs_kv[:])
            nc.gpsimd.dma_start(msc_kv_t[:], msc_kv[:])
            xkv_t = cmp_x.tile([128, 16, 512], dtA, name="xkv_t")
            xq_t = cmp_x.tile([128, 16, LQ], dtA, name="xq_t")
            wcq_t = cmp_w.tile([128, 16, QR], dtA, name="wcq_t")
            wckx_t = cmp_w.tile([128, 16, 640], dtA, name="wckx_t")
            for k in range(16):
                nc.sync.dma_start(wckx_t[:, k, :], w_ckx[:, k, :])
                nc.sync.dma_start(xkv_t[:, k, :], x_kv[:, k, :])
            for k in range(16):
                nc.sync.dma_start(wcq_t[:, k, :], w_cq[:, k, :])
                nc.sync.dma_start(xq_t[:, k, :], x_q[:, k, :])

            kr_sb = cmp_t.tile([64, 512], dtA, name="kr_sb", bufs=1)
            # supertile order kv0, q, kv1: AG0 ships ASAP, q-path work then
            # fills the PE while the collectives run
            for sup in (0, 2, 1):
                is_q = sup == 2
                xs = slice(0, LQ) if is_q else slice(sup * 256, (sup + 1) * 256)
                x_t = xq_t if is_q else xkv_t
                w_t = wcq_t if is_q else wckx_t
                nblk = 4 if is_q else 5
                nf = 512 if is_q else 256
                pss = [ps_cmp.tile([128, 512], FP32, name=f"ps_c{b}", tag=f"ps_c{b}")
                       for b in range(nblk)]
                for k in range(16):
                    for blk in range(nblk):
                        nc.tensor.matmul(
                            pss[blk][:, 0:nf], w_t[:, k, blk * 128:(blk + 1) * 128],
                            x_t[:, k, xs], start=(k == 0), stop=(k == 15))
                lat_raw = cmp_t.tile([128, 4, 512], FP16, name="lat_raw", tag="lat_raw")
                sq_t = cmp_t.tile([128, 4, 512], FP16, name="sq_t", tag="sq_t")
                for blk in range(4):
                    cp(lat_raw[:, blk, 0:nf], pss[blk][:, 0:nf])
                    nc.vector.tensor_mul(sq_t[:, blk, 0:nf], lat_raw[:, blk, 0:nf],
                                         lat_raw[:, blk, 0:nf])
                if not is_q:
                    # krope block: rows 0:64 p2, 64:128 r2 (cols 512:640)
                    u2 = cmp_t.tile([64, 512], FP16, name="u2", tag="u2")
                    v2 = cmp_t.tile([64, 512], FP16, name="v2", tag="v2")
                    nc.vector.tensor_mul(u2[:, 0:nf], pss[4][0:64, 0:nf],
                                         cs_kv_t[:, xs])
                    nc.vector.tensor_mul(v2[:, 0:nf], pss[4][64:128, 0:nf],
                                         msc_kv_t[:, xs])
                    nc.vector.tensor_add(kr_sb[:, xs], u2[:, 0:nf], v2[:, 0:nf])
                # rmsnorm: ssq -> rstd -> broadcast -> scale
                ps_ssq = ps_nrm.tile([1, 512], FP32, name="ps_ssq", tag="ps_ssq")
                for blk in range(4):
                    nc.tensor.matmul(ps_ssq[:, 0:nf], invn[:], sq_t[:, blk, 0:nf],
                                     start=(blk == 0), stop=(blk == 3))
                std_f = cmp_t.tile([1, 512], FP32, name="std_f", tag="std_f")
                nc.scalar.activation(std_f[:, 0:nf], ps_ssq[:, 0:nf], AF.Sqrt,
                                     bias=eps_t[:])
                rstd_f = cmp_t.tile([1, 512], FP16, name="rstd_f", tag="rstd_f")
                with nc.allow_low_precision(reason="rstd is O(1); fp16 ok"):
                    nc.vector.reciprocal(rstd_f[:, 0:nf], std_f[:, 0:nf])
                ps_rb = ps_nrm.tile([128, 512], FP32, name="ps_rb", tag="ps_rb")
                nc.tensor.matmul(ps_rb[:, 0:nf], ones1[:], rstd_f[:, 0:nf],
                                 start=True, stop=True)
                rstd_sb = cmp_t.tile([128, 512], FP16, name="rstd_sb", tag="rstd_sb")
                cp(rstd_sb[:, 0:nf], ps_rb[:, 0:nf])
                kvl_sb = cmp_t.tile([128, 4, 512], dtA, name="kvl_sb", tag="kvl_sb")
                if is_q:
                    for blk in range(4):
                        nc.vector.tensor_mul(qlat[:, blk, :],
                                             lat_raw[:, blk, :], rstd_sb[:])
                else:
                    for blk in range(4):
                        nc.vector.tensor_mul(kvl_sb[:, blk, 0:nf],
                                             lat_raw[:, blk, 0:nf], rstd_sb[:, 0:nf])
                    # ship this half: latent + krope -> DRAM -> AllGather
                    t = sup
                    nc.scalar.dma_start(
                        lat_kin[t][0:512, :].rearrange("(c p) n -> p c n", p=128),
                        kvl_sb[:, :, 0:nf])
                    nc.scalar.dma_start(lat_kin[t][512:576, :], kr_sb[:, xs])
                    nc.gpsimd.collective_compute(
                        "AllGather",
                        mybir.AluOpType.bypass,
                        ins=[lat_kin[t][:].rearrange("a b -> (a b)")],
                        outs=[lat_g[t][:].rearrange("w a b -> (w a b)")],
                        replica_groups=[[g * 4 + i for i in range(4)]
                                        for g in range(2)],
                    )

            # load gathered halves (gathered order; attention uses GPERM)
            for t in range(2):
                for c in range(4):
                    nc.gpsimd.dma_start(
                        kvlat_h[t][:, c, :].rearrange("p (w m) -> p w m", w=4),
                        lat_g[t][:, c * 128:(c + 1) * 128, :]
                        .rearrange("w p m -> p w m"))
                nc.gpsimd.dma_start(
                    krope_h[t][0:64, :].rearrange("p (w m) -> p w m", w=4),
                    lat_g[t][:, 512:576, :].rearrange("w p m -> p w m"))
                nc.gpsimd.dma_start(krope_h[t][64:128, :], krope_h[t][0:64, :])

        if DEBUG:
            for t in range(2):
                nc.sync.dma_start(dbg_kvlat[:, :, t * 1024:(t + 1) * 1024],
                                  kvlat_h[t][:])
                nc.sync.dma_start(dbg_krope[:, t * 1024:(t + 1) * 1024],
                                  krope_h[t][:])
            nc.sync.dma_start(dbg_qlat[:], qlat[:])

        ap_cm = tc.tile_pool(name="att_persist", bufs=1)
        ap_pool = ap_cm.__enter__()
        attn_out = ap_pool.tile([128, H, LQ], FP16, name="attn_out")
        mask_t = ap_pool.tile([128, 16, QW], FP16, name="mask_t")
        nc.sync.dma_start(mask_t[:], masks[:])

        # q decompress for all 16 heads up front (only needs qlat; fills the
        # PE while the latent AllGathers are in flight)
        qn_all = ap_pool.tile([128, H, LQ], FP16, name="qn_all")
        qr2_all = ap_pool.tile([128, H // 2, LQ], FP16, name="qr2_all")
        if not fp8:
            with tc.tile_pool(name="qdec_w", bufs=1) as qdec_w, \
                 tc.tile_pool(name="qdec_t", bufs=2) as qdec_t, \
                 tc.tile_pool(name="ps_qd", bufs=3, space="PSUM") as ps_qd:
                wdqn_t = qdec_w.tile([128, 4, H * 128], dtA, name="wdqn_t")
                wdqr2_t = qdec_w.tile([128, 4, H * 128], dtA, name="wdqr2_t")
                nc.sync.dma_start(wdqn_t[:], w_dqn[:])
                nc.sync.dma_start(wdqr2_t[:], w_dqr2[:])
                for h in range(H):
                    ps = ps_qd.tile([128, 512], FP32, name="ps_qn", tag="ps_q")
                    contract(ps[:], wdqn_t, 0, slice(h * 128, (h + 1) * 128),
                             qlat, 0, slice(0, LQ), 4)
                    cp(qn_all[:, h, :], ps[:])
                for pr in range(H // 2):
                    psp = ps_qd.tile([128, 512], FP32, name="ps_p2", tag="ps_q")
                    psr = ps_qd.tile([128, 512], FP32, name="ps_r2", tag="ps_q")
                    contract(psp[:], wdqr2_t, 0,
                             slice(pr * 256, pr * 256 + 128),
                             qlat, 0, slice(0, LQ), 4)
                    contract(psr[:], wdqr2_t, 0,
                             slice(pr * 256 + 128, pr * 256 + 256),
                             qlat, 0, slice(0, LQ), 4)
                    u2 = qdec_t.tile([128, 512], FP16, name="qu2", tag="qu2")
                    v2 = qdec_t.tile([128, 512], FP16, name="qv2", tag="qv2")
                    nc.vector.tensor_mul(u2[:], psp[:], cs_q2_t[:])
                    nc.vector.tensor_mul(v2[:], psr[:], msc_q2_t[:])
                    nc.vector.tensor_add(qr2_all[:, pr, :], u2[:], v2[:])

        # ---------------- Phase B: per head-pass decompress + attention ------
        for hp in range(NPASS):
            hbase = hp * HPP
            with tc.tile_pool(name="dec_w", bufs=1) as dec_w, \
                 tc.tile_pool(name="kv_sb", bufs=1) as kv_sb, \
                 tc.tile_pool(name="q_sb", bufs=1) as q_sb:
                dec_cm = tc.tile_pool(name="dec_t", bufs=2)
                dec_t = dec_cm.__enter__()
                psd_cm = tc.tile_pool(name="ps_dec", bufs=2, space="PSUM")
                ps_dec = psd_cm.__enter__()
                hsl = slice(hbase * 128, (hbase + HPP) * 128)
                wdkn_t = dec_w.tile([128, 4, HPP * 128], dtA, name="wdkn_t")
                wdv_t = dec_w.tile([128, 4, HPP * 128], dtA, name="wdv_t")
                nc.sync.dma_start(wdkn_t[:], w_dkn[:, :, hsl])
                nc.sync.dma_start(wdv_t[:], w_dv[:, :, hsl])
                if fp8:
                    raise NotImplementedError("fp8 path with hoisted qdec")

                # --- kv decompress (8 heads, all 2048 keys) ---
                # v dtype must match pTs dtype in the AV matmul (no mixed
                # fp8 x fp16 matmuls).
                if fp8:
                    knv2 = kv_sb.tile([128, HPP, 2, S], FP8, name="knv2")
                else:
                    k_n = kv_sb.tile([128, HPP, S], FP16, name="k_n")
                v_t = kv_sb.tile([128, NKT, HPP * VH],
                                 FP8 if fp8pt else FP16, name="v_t")
                def kvdec_half(tg):
                    lat = kvlat_h[tg]
                    for ksl in range(2):
                        ks = slice(ksl * 512, (ksl + 1) * 512)
                        ksg = slice((2 * tg + ksl) * 512, (2 * tg + ksl + 1) * 512)
                        for h in range(HPP):
                            ps = ps_dec.tile([128, 512], FP32, name="ps_kn",
                                             tag="ps_d")
                            contract(ps[:], wdkn_t, 0,
                                     slice(h * 128, (h + 1) * 128),
                                     lat, 0, ks, 4)
                            if fp8:
                                cp(knv2[:, h, 0, ksg], ps[:])
                            else:
                                cp(k_n[:, h, ksg], ps[:])
                    for ktl in range(8):
                        kt = 8 * tg + ktl
                        for g in range(2):      # head groups of 4
                            ps = ps_dec.tile([128, 512], FP32, name="ps_v",
                                             tag="ps_d")
                            for i in range(4):
                                nc.tensor.matmul(
                                    ps[:], lat[:, i, ktl * 128:(ktl + 1) * 128],
                                    wdv_t[:, i, g * 512:(g + 1) * 512],
                                    start=(i == 0), stop=(i == 3))
                            cp(v_t[:, kt, g * 512:(g + 1) * 512], ps[:])

                if DEBUG and hp == 0 and not fp8:
                    nc.sync.dma_start(dbg_kn[:], k_n[:])
                    nc.sync.dma_start(dbg_vt[:], v_t[:])
                    nc.sync.dma_start(dbg_qn[:], qn_t[:])
                    nc.sync.dma_start(dbg_qr2[:], qr2_t[:])
                if DEBUG and hp == 0 and fp8:
                    nc.sync.dma_start(dbg_knv2[:], knv2[:])
                    nc.sync.dma_start(dbg_qdr[:], qdr_t[:])
                    nc.sync.dma_start(dbg_vt8[:], v_t[:])

                kvdec_half(0)
                ps_att_cm = tc.tile_pool(name="ps_att", bufs=2, space="PSUM")
                ps_att = ps_att_cm.__enter__()
                ps_avz_cm = tc.tile_pool(name="ps_avz", bufs=2, space="PSUM")
                ps_avz = ps_avz_cm.__enter__()
                att_cm = tc.tile_pool(name="att_t", bufs=2)
                att_t = att_cm.__enter__()

                # --- attention: 8 slots, slot s has 2s+2 key tiles ---
                # pav accumulation groups at F-offsets of one psum bank must
                # be sequential per head (interleaving breaks psum), so keep
                # the whole slot's exp'd probabilities in SBUF, then run each
                # head's AV as one contiguous accumulation group.
                def emit_av(st):
                    s_, pav_, pz_, pTs_ = st
                    trip_ = 2 * s_ + 2
                    qs_ = slice(s_ * QW, (s_ + 1) * QW)
                    for h in range(HPP):
                        fs = slice(h * QW, (h + 1) * QW)
                        vs = slice(h * VH, (h + 1) * VH)
                        if fp8pt:
                            raise NotImplementedError("fp8pt with GPERM")
                        else:
                            for r in range(trip_):
                                nc.tensor.matmul(
                                    pav_[:, fs], v_t[:, GPERM[r], vs],
                                    pTs_[:, r, fs],
                                    start=(r == 0), stop=(r == trip_ - 1))
                    rz = att_t.tile([128, 512], FP32, name="rz", tag="rz")
                    nc.vector.reciprocal(rz[:], pz_[:])
                    nc.vector.tensor_mul(
                        attn_out[:, hbase:hbase + HPP, qs_],
                        pav_[:].rearrange("p (h q) -> p h q", h=HPP),
                        rz[:].rearrange("p (h q) -> p h q", h=HPP))

                pend = None
                for s in range(NSLOT):
                    if s == 4:
                        kvdec_half(1)
                    trip = 2 * s + 2
                    qs = slice(s * QW, (s + 1) * QW)
                    pav = ps_avz.tile([128, 512], FP32, name="pav", tag="pav")
                    pz = ps_avz.tile([128, 512], FP32, name="pz", tag="pz")
                    pTs = att_t.tile([128, NKT, 512], FP8E5 if fp8pt else FP16,
                                     name="pTs", tag="pTs", bufs=2)
                    for r in range(trip):
                        g = GPERM[r]
                        ks = slice(g * 128, (g + 1) * 128)
                        sc = ps_att.tile([128, 512], FP32, name="sc", tag="sc",
                                         bufs=2)
                        for h in range(HPP):
                            fs = slice(h * QW, (h + 1) * QW)
                            if fp8:
                                nc.tensor.matmul(
                                    sc[:, fs], knv2[:, h, :, ks],
                                    qdr_t[:, 2 * h:2 * h + 2, 0, qs],
                                    start=True, stop=True, perf_mode=DRM)
                            else:
                                hg = hbase + h
                                nc.tensor.matmul(sc[:, fs], k_n[:, h, ks],
                                                 qn_all[:, hg, qs],
                                                 start=True, stop=False)
                                ro = (h % 2) * 64
                                nc.tensor.matmul(
                                    sc[:, fs],
                                    krope_h[g // 8][ro:ro + 64,
                                                    (g % 8) * 128:(g % 8) * 128 + 128],
                                    qr2_all[ro:ro + 64, hg // 2, qs],
                                    start=False, stop=True)
                        if r >= 2 * s:      # masked iters (diag + pad)
                            m = 2 * s + (r - 2 * s)
                            scv = sc[:].rearrange("p (h q) -> p h q", h=HPP)
                            mb = mask_t[:, m, :].unsqueeze(1).broadcast_to(
                                [128, HPP, QW])
                            nc.vector.tensor_add(scv, scv, mb)
                        nc.scalar.activation(pTs[:, r, :], sc[:], AF.Exp,
                                             bias=zero1[:])
                        # pz accumulates the full bank (safe to interleave
                        # with sc groups); deferred one iter so the PE never
                        # waits on the exp it just requested.
                        if not fp8pt and r > 0:
                            nc.tensor.matmul(pz[:], ones_h[:], pTs[:, r - 1, :],
                                             start=(r == 1), stop=False)
                        if r == 1 and pend is not None:
                            emit_av(pend)
                            pend = None
                    nc.tensor.matmul(pz[:], ones_h[:], pTs[:, trip - 1, :],
                                     start=False, stop=True)
                    pend = (s, pav, pz, pTs)
                    if DEBUG and hp == 0 and s == 0 and not fp8:
                        nc.sync.dma_start(dbg_pt[:], pTs[:, 0:2, :])
                if pend is not None:
                    emit_av(pend)
                    pend = None
                att_cm.__exit__(None, None, None)
                ps_avz_cm.__exit__(None, None, None)
                ps_att_cm.__exit__(None, None, None)
                psd_cm.__exit__(None, None, None)
                dec_cm.__exit__(None, None, None)

        if DEBUG:
            nc.sync.dma_start(dbg_ao[:], attn_out[:])

        # ---------------- Phase C: output projection -------------------------
        with tc.tile_pool(name="prj_w", bufs=3) as prj_w, \
             tc.tile_pool(name="prj_t", bufs=3) as prj_t, \
             tc.tile_pool(name="ps_prj", bufs=3, space="PSUM") as ps_prj:
            for ob in range(16):
                wp = prj_w.tile([128, 16, 128], FP16, name="wp", tag="wp")
                nc.sync.dma_start(wp[:], w_proj[:, :, ob * 128:(ob + 1) * 128])
                ps = ps_prj.tile([128, 512], FP32, name="ps_o", tag="ps_o")
                for h in range(16):
                    nc.tensor.matmul(ps[:], wp[:, h, :], attn_out[:, h, :],
                                     start=(h == 0), stop=(h == 15))
                ot = prj_t.tile([128, 512], FP32, name="ot", tag="ot")
                cp(ot[:], ps[:])
                nc.scalar.dma_start(out_c[:, ob, :], ot[:])

        ap_cm.__exit__(None, None, None)
        lat_cm.__exit__(None, None, None)
        dram_cm.__exit__(None, None, None)
        const_cm.__exit__(None, None, None)

    nc.compile()
    return nc


def _qsel(j):
    """Local q token order for core with within-batch index j."""
    idx = []
    for s in range(NSLOT):
        t = 4 * s + 3 - j
        idx.extend(range(t * QW, (t + 1) * QW))
    return np.array(idx)


def _prep_inputs(x, freqs_cis, w_cq, w_qnorm, w_dqn, w_dqr, w_ckv, w_kvnorm,
                 w_dkn, w_dv, w_krope, w_proj):
    import ml_dtypes
    f16 = np.float16
    fA = ml_dtypes.float8_e4m3fn if USE_FP8 else f16
    perm = np.concatenate([np.arange(0, ROPE, 2), np.arange(1, ROPE, 2)])
    pe, po = perm[:HALF], perm[HALF:]

    def chunk_major(a, nch):
        # [K, C] -> [128, nch, C] with K = 128*nch
        return np.ascontiguousarray(
            a.reshape(nch, 128, a.shape[1]).transpose(1, 0, 2))

    # compress weights (lhsT layout [K=D, P=out])
    wcq_l = chunk_major(w_cq.T.astype(fA), 16)                    # [128,16,512]
    wkr = (w_krope / H)                                           # [64, D]
    ckx = np.concatenate([w_ckv, wkr[pe], wkr[pe], wkr[po], wkr[po]], axis=0)
    # krope block rows 512:640: p2 = [even;even], r2 = [odd;odd]
    wckx_l = chunk_major(ckx.T.astype(fA), 16)                    # [128,16,640]

    # decompress weights, norm + scale folded
    dqn = (w_dqn * w_qnorm[None, :] * SCALE)                      # [H*128, QR]
    wdqn_l = chunk_major(dqn.T.astype(fA), 4)                     # [128,4,2048]
    dqr = (w_dqr * w_qnorm[None, :] * SCALE).reshape(H, ROPE, QR)
    dqr2 = np.empty((H // 2, 4, HALF * 2, QR), np.float32)
    for p in range(H // 2):
        h0, h1 = 2 * p, 2 * p + 1
        # rows: [x0;x0] for p2 blocks, [x1;x1] for r2 blocks
        dqr2[p, 0, :HALF] = dqr[h0][pe]; dqr2[p, 0, HALF:] = dqr[h0][pe]
        dqr2[p, 1, :HALF] = dqr[h1][pe]; dqr2[p, 1, HALF:] = dqr[h1][pe]
        dqr2[p, 2, :HALF] = dqr[h0][po]; dqr2[p, 2, HALF:] = dqr[h0][po]
        dqr2[p, 3, :HALF] = dqr[h1][po]; dqr2[p, 3, HALF:] = dqr[h1][po]
    # layout per pair: cols [p2_h0(64) p2_h1(64) r2_h0(64) r2_h1(64)]
    dqr2 = dqr2.reshape(H // 2 * 4 * ROPE, QR)                    # [2048, 512]
    wdqr2_l = chunk_major(np.ascontiguousarray(dqr2.T).astype(fA), 4)
    dkn = (w_dkn * w_kvnorm[None, :])
    wdkn_l = chunk_major(dkn.T.astype(fA), 4)
    dvw = (w_dv * w_kvnorm[None, :])
    wdv_l = chunk_major(dvw.T.astype(fA), 4)
    wproj_l = chunk_major(np.ascontiguousarray(w_proj.T).astype(f16), 16)

    cos = freqs_cis[:, :, 0].T.astype(np.float32)                 # [32, S]
    sin = freqs_cis[:, :, 1].T.astype(np.float32)
    cs_kv = np.concatenate([cos, sin], 0).astype(f16)             # [64, S]
    msc_kv = np.concatenate([-sin, cos], 0).astype(f16)

    xT = [np.ascontiguousarray(x[b].T) for b in range(B)]         # [D, S]

    in_maps = []
    for c in range(W):
        b, j = c // 4, c % 4
        qsel = _qsel(j)
        kvsel = np.concatenate([np.arange(128 * (4 * k + j), 128 * (4 * k + j) + 128)
                                for k in range(4)])
        xkv_l = chunk_major(np.ascontiguousarray(xT[b][:, kvsel]).astype(fA), 16)
        xq_l = chunk_major(np.ascontiguousarray(xT[b][:, qsel]).astype(fA), 16)
        csq = cs_kv[:, qsel]
        mscq = msc_kv[:, qsel]
        cs_q2 = np.concatenate([csq, csq], 0)                     # [128, LQ]
        msc_q2 = np.concatenate([mscq, mscq], 0)
        # masks: slot s, d in {0,1} -> iter r = 2s+d, additive 0/-10000
        mk = np.zeros((128, 16, QW), np.float32)
        for s in range(NSLOT):
            for d_ in range(2):
                r = 2 * s + d_
                kg = r * 128 + np.arange(128)
                qg = qsel[s * QW:(s + 1) * QW]
                mk[:, 2 * s + d_, :] = np.where(qg[None, :] >= kg[:, None],
                                                0.0, -10000.0)
        in_maps.append({
            "x_kv": xkv_l, "x_q": xq_l,
            "w_cq": wcq_l, "w_ckx": wckx_l,
            "w_dqn": wdqn_l, "w_dqr2": wdqr2_l, "w_dkn": wdkn_l, "w_dv": wdv_l,
            "w_proj": wproj_l,
            "cs_kv": np.ascontiguousarray(cs_kv[:, kvsel]),
            "msc_kv": np.ascontiguousarray(msc_kv[:, kvsel]),
            "cs_q2": cs_q2.astype(f16), "msc_q2": msc_q2.astype(f16),
            "masks": mk.astype(f16),
        })
    return in_maps


last_results = None


def kernel(x, mask, freqs_cis, w_cq, w_qnorm, w_dqn, w_dqr, w_ckv, w_kvnorm,
           w_dkn, w_dv, w_krope, w_proj):
    global last_results
    if "nc" not in _cache:
        _cache["nc"] = _build()
    nc = _cache["nc"]

    args = [np.asarray(a, np.float32) for a in
            (x, freqs_cis, w_cq, w_qnorm, w_dqn, w_dqr, w_ckv, w_kvnorm,
             w_dkn, w_dv, w_krope, w_proj)]
    in_maps = _prep_inputs(*args)

    res = bass_utils.run_bass_kernel_spmd(nc, in_maps, core_ids=list(range(W)))
    last_results = res

    out = np.empty((B, S, D), np.float32)
    for c in range(W):
        b, j = c // 4, c % 4
        oc = res.results[c]["out_c"]          # [128, 16, 512]
        flat = oc.transpose(1, 0, 2).reshape(D, LQ)
        out[b, _qsel(j), :] = flat.T
    return out


# revision 4
# speedup vs baseline: 1.0252x; 1.0252x over previous
"""MLA forward on 8 Trainium2 NeuronCores — zero-collective design.

Each core owns one batch (4 cores per batch) and 512 query tokens arranged as
8 slots of 64, chosen so every core's causal attention has identical shape
(slot s attends 2s+2 key-tiles of 128). The kv path (compress + decompress of
all 16 heads for the full 2048 positions) is replicated across the batch's 4
cores, which removes all collectives. Output projection is local to each
core's tokens; the host reassembles.
"""

import numpy as np

import concourse.bacc as bacc
import concourse.bass_isa as bass_isa
import concourse.mybir as mybir
import concourse.tile as tile
from concourse import bass_utils

B, S, D = 2, 2048, 2048
H = 16
NOPE, ROPE, VH = 128, 64, 128
HALF = ROPE // 2
QR = KVR = 512
EPS = 1e-6
W = 8                      # cores
LQ = 512                   # local q tokens per core
NSLOT = 8                  # q slots of 64
QW = 64
NKT = 16                   # key tiles of 128
SCALE = 1.0 / np.sqrt(NOPE + ROPE)
HPP = 8                    # heads per pass
NPASS = 2
# ownership: key tile b owned by core b%4; AG half t carries the owner's
# tile-slots {2t, 2t+1}; natural tile b sits at gathered block GPERM[b].
GPERM = [8 * ((b // 4) // 2) + 2 * (b % 4) + ((b // 4) % 2) for b in range(16)]

FP16 = mybir.dt.float16
FP32 = mybir.dt.float32
FP8 = mybir.dt.float8e4
FP8E5 = mybir.dt.float8e5
DRM = mybir.MatmulPerfMode.DoubleRow
AF = mybir.ActivationFunctionType

DEBUG = False
USE_FP8 = False            # compress/decompress/scores path in fp8-DoubleRow
USE_FP8_PT = False         # attention probabilities in fp8e5 (AV/pz DoubleRow)

_cache = {}


def _build():
    fp8 = USE_FP8
    fp8pt = USE_FP8_PT and fp8
    dtA = FP8 if fp8 else FP16
    dtP = FP8E5 if fp8pt else FP16

    nc = bacc.Bacc("TRN2", target_bir_lowering=False, debug=False)

    def din(name, shape, dt=FP16):
        return nc.dram_tensor(name, shape, dt, kind="ExternalInput").ap()

    x_kv = din("x_kv", [128, 16, 512], dtA)   # this core's kv positions (p%4==j)
    x_q = din("x_q", [128, 16, LQ], dtA)
    w_cq = din("w_cq", [128, 16, QR], dtA)
    w_ckx = din("w_ckx", [128, 16, 640], dtA)      # ckv 512 | krope p2 64 | krope r2 64
    w_dqn = din("w_dqn", [128, 4, H * NOPE], dtA)
    w_dqr2 = din("w_dqr2", [128, 4, H * 128], dtA)  # per head-pair: p2|p2|r2|r2 blocks
    w_dkn = din("w_dkn", [128, 4, H * NOPE], dtA)
    w_dv = din("w_dv", [128, 4, H * VH], dtA)
    w_proj = din("w_proj", [128, 16, D])
    cs_kv = din("cs_kv", [64, 512])
    msc_kv = din("msc_kv", [64, 512])
    cs_q2 = din("cs_q2", [128, LQ])
    msc_q2 = din("msc_q2", [128, LQ])
    masks = din("masks", [128, 16, QW])             # additive 0/-10000 per slot/iter
    out_c = nc.dram_tensor("out_c", [128, 16, LQ], FP32, kind="ExternalOutput").ap()
    if DEBUG:
        dbg_kvlat = nc.dram_tensor("dbg_kvlat", [128, 4, S], dtA, kind="ExternalOutput").ap()
        dbg_qlat = nc.dram_tensor("dbg_qlat", [128, 4, LQ], dtA, kind="ExternalOutput").ap()
        dbg_krope = nc.dram_tensor("dbg_krope", [128, S], dtA, kind="ExternalOutput").ap()
        dbg_kn = nc.dram_tensor("dbg_kn", [128, HPP, S], FP16, kind="ExternalOutput").ap()
        dbg_vt = nc.dram_tensor("dbg_vt", [128, NKT, HPP * VH], FP16, kind="ExternalOutput").ap()
        dbg_qn = nc.dram_tensor("dbg_qn", [128, HPP, LQ], FP16, kind="ExternalOutput").ap()
        dbg_qr2 = nc.dram_tensor("dbg_qr2", [128, HPP // 2, LQ], FP16, kind="ExternalOutput").ap()
        dbg_ao = nc.dram_tensor("dbg_ao", [128, H, LQ], FP16, kind="ExternalOutput").ap()
        dbg_pt = nc.dram_tensor("dbg_pt", [128, 2, 512], FP16, kind="ExternalOutput").ap()
        dbg_knv2 = nc.dram_tensor("dbg_knv2", [128, HPP, 2, S], FP8, kind="ExternalOutput").ap()
        dbg_qdr = nc.dram_tensor("dbg_qdr", [128, 2 * HPP, 2, LQ], FP8, kind="ExternalOutput").ap()
        dbg_vt8 = nc.dram_tensor("dbg_vt8", [128, NKT, HPP * VH],
                                 FP8 if (USE_FP8 and USE_FP8_PT) else FP16,
                                 kind="ExternalOutput").ap()
        dbg_pav = nc.dram_tensor("dbg_pav", [128, 512], FP32, kind="ExternalOutput").ap()
        dbg_pz = nc.dram_tensor("dbg_pz", [128, 512], FP32, kind="ExternalOutput").ap()

    nch = 2 if fp8 else 1   # contraction step: chunks consumed per matmul

    def contract(ps_ap, w_t, wc, wsl, r_t, rc, rsl, n):
        """Accumulate ps += sum over n chunks: w[:, c, wsl].T @ r[:, c, rsl].
        wc/rc index the chunk dim of 3D tiles w_t/r_t."""
        if fp8:
            for i in range(n // 2):
                nc.tensor.matmul(
                    ps_ap,
                    w_t[:, wc + 2 * i:wc + 2 * i + 2, wsl],
                    r_t[:, rc + 2 * i:rc + 2 * i + 2, rsl],
                    start=(i == 0), stop=(i == n // 2 - 1), perf_mode=DRM)
        else:
            for i in range(n):
                nc.tensor.matmul(
                    ps_ap, w_t[:, wc + i, wsl], r_t[:, rc + i, rsl],
                    start=(i == 0), stop=(i == n - 1))

    cp_engines = None

    def cp(dst, src):
        cp_engines.append(cp_engines.pop(0))
        eng = cp_engines[-1]
        if eng is nc.scalar:
            eng.copy(dst, src)
        else:
            eng.tensor_copy(dst, src)

    with tile.TileContext(nc) as tc:
        cp_engines = [nc.vector, nc.scalar]

        const_cm = tc.tile_pool(name="const", bufs=1)
        const = const_cm.__enter__()
        zero1 = const.tile([128, 1], FP32, name="zero1")
        nc.any.memset(zero1[:], 0.0)
        eps_t = const.tile([1, 1], FP32, name="eps_t")
        nc.any.memset(eps_t[:], EPS)
        invn = const.tile([128, 1], FP16, name="invn")
        nc.any.memset(invn[:], 1.0 / QR)
        ones1 = const.tile([1, 128], FP16, name="ones1")
        nc.any.memset(ones1[:], 1.0)
        if fp8pt:
            ones_h = const.tile([128, 2, 128], FP8, name="ones_h")
        else:
            ones_h = const.tile([128, 128], FP16, name="ones_h")
        nc.any.memset(ones_h[:], 1.0)
        cs_q2_t = const.tile([128, LQ], FP16, name="cs_q2_t")
        msc_q2_t = const.tile([128, LQ], FP16, name="msc_q2_t")
        nc.gpsimd.dma_start(cs_q2_t[:], cs_q2[:])
        nc.gpsimd.dma_start(msc_q2_t[:], msc_q2[:])

        # persistent activations
        lat_cm = tc.tile_pool(name="lat", bufs=1)
        lat_pool = lat_cm.__enter__()
        kvlat_h = [lat_pool.tile([128, 4, S // 2], dtA, name=f"kvlat{t}")
                   for t in range(2)]
        qlat = lat_pool.tile([128, 4, LQ], dtA, name="qlat")
        krope_h = [lat_pool.tile([128, S // 2], dtA, name=f"krope{t}")
                   for t in range(2)]

        # ---------------- Phase A: compress own positions + AllGather --------
        # Each core compresses only its own 512 kv positions (p % 4 == j in
        # its batch), then the 4 batch cores AllGather the latents+krope in
        # two pipelined halves. The gathered loads undo the position
        # interleave so decompress sees natural key order.
        dram_cm = tc.tile_pool(name="dram", bufs=1, space="DRAM")
        dram = dram_cm.__enter__()
        lat_kin = [dram.tile([576, 256], dtA, tag=f"lat_kin{t}",
                             name=f"lat_kin{t}") for t in range(2)]
        lat_g = [dram.tile([4, 576, 256], dtA, tag=f"lat_g{t}",
                           name=f"lat_g{t}") for t in range(2)]

        with tc.tile_pool(name="cmp_x", bufs=1) as cmp_x, \
             tc.tile_pool(name="cmp_w", bufs=1) as cmp_w, \
             tc.tile_pool(name="cmp_t", bufs=2) as cmp_t, \
             tc.tile_pool(name="ps_cmp", bufs=1, space="PSUM") as ps_cmp, \
             tc.tile_pool(name="ps_nrm", bufs=1, space="PSUM") as ps_nrm:
            cs_kv_t = cmp_x.tile([64, 512], FP16, name="cs_kv_t")
            msc_kv_t = cmp_x.tile([64, 512], FP16, name="msc_kv_t")
            nc.gpsimd.dma_start(cs_kv_t[:], cs_kv[:])
            nc.gpsimd.dma_start(msc_kv_t[:], msc_kv[:])
            xkv_t = cmp_x.tile([128, 16, 512], dtA, name="xkv_t")
            xq_t = cmp_x.tile([128, 16, LQ], dtA, name="xq_t")
            wcq_t = cmp_w.tile([128, 16, QR], dtA, name="wcq_t")
            wckx_t = cmp_w.tile([128, 16, 640], dtA, name="wckx_t")
            for k in range(16):
                nc.sync.dma_start(wckx_t[:, k, :], w_ckx[:, k, :])
                nc.sync.dma_start(xkv_t[:, k, :], x_kv[:, k, :])
            for k in range(16):
                nc.sync.dma_start(wcq_t[:, k, :], w_cq[:, k, :])
                nc.sync.dma_start(xq_t[:, k, :], x_q[:, k, :])

            kr_sb = cmp_t.tile([64, 512], dtA, name="kr_sb", bufs=1)
            # supertile order kv0, q, kv1: AG0 ships ASAP, q-path work then
            # fills the PE while the collectives run
            for sup in (0, 2, 1):
                is_q = sup == 2
                xs = slice(0, LQ) if is_q else slice(sup * 256, (sup + 1) * 256)
                x_t = xq_t if is_q else xkv_t
                w_t = wcq_t if is_q else wckx_t
                nblk = 4 if is_q else 5
                nf = 512 if is_q else 256
                pss = [ps_cmp.tile([128, 512], FP32, name=f"ps_c{b}", tag=f"ps_c{b}")
                       for b in range(nblk)]
                for k in range(16):
                    for blk in range(nblk):
                        nc.tensor.matmul(
                            pss[blk][:, 0:nf], w_t[:, k, blk * 128:(blk + 1) * 128],
                            x_t[:, k, xs], start=(k == 0), stop=(k == 15))
                lat_raw = cmp_t.tile([128, 4, 512], FP16, name="lat_raw", tag="lat_raw")
                sq_t = cmp_t.tile([128, 4, 512], FP16, name="sq_t", tag="sq_t")
                for blk in range(4):
                    cp(lat_raw[:, blk, 0:nf], pss[blk][:, 0:nf])
                    nc.vector.tensor_mul(sq_t[:, blk, 0:nf], lat_raw[:, blk, 0:nf],
                                         lat_raw[:, blk, 0:nf])
                if not is_q:
                    # krope block: rows 0:64 p2, 64:128 r2 (cols 512:640)
                    u2 = cmp_t.tile([64, 512], FP16, name="u2", tag="u2")
                    v2 = cmp_t.tile([64, 512], FP16, name="v2", tag="v2")
                    nc.vector.tensor_mul(u2[:, 0:nf], pss[4][0:64, 0:nf],
                                         cs_kv_t[:, xs])
                    nc.vector.tensor_mul(v2[:, 0:nf], pss[4][64:128, 0:nf],
                                         msc_kv_t[:, xs])
                    nc.vector.tensor_add(kr_sb[:, xs], u2[:, 0:nf], v2[:, 0:nf])
                # rmsnorm: ssq -> rstd -> broadcast -> scale
                ps_ssq = ps_nrm.tile([1, 512], FP32, name="ps_ssq", tag="ps_ssq")
                for blk in range(4):
                    nc.tensor.matmul(ps_ssq[:, 0:nf], invn[:], sq_t[:, blk, 0:nf],
                                     start=(blk == 0), stop=(blk == 3))
                std_f = cmp_t.tile([1, 512], FP32, name="std_f", tag="std_f")
                nc.scalar.activation(std_f[:, 0:nf], ps_ssq[:, 0:nf], AF.Sqrt,
                                     bias=eps_t[:])
                rstd_f = cmp_t.tile([1, 512], FP16, name="rstd_f", tag="rstd_f")
                with nc.allow_low_precision(reason="rstd is O(1); fp16 ok"):
                    nc.vector.reciprocal(rstd_f[:, 0:nf], std_f[:, 0:nf])
                ps_rb = ps_nrm.tile([128, 512], FP32, name="ps_rb", tag="ps_rb")
                nc.tensor.matmul(ps_rb[:, 0:nf], ones1[:], rstd_f[:, 0:nf],
                                 start=True, stop=True)
                rstd_sb = cmp_t.tile([128, 512], FP16, name="rstd_sb", tag="rstd_sb")
                cp(rstd_sb[:, 0:nf], ps_rb[:, 0:nf])
                kvl_sb = cmp_t.tile([128, 4, 512], dtA, name="kvl_sb", tag="kvl_sb")
                if is_q:
                    for blk in range(4):
                        nc.vector.tensor_mul(qlat[:, blk, :],
                                             lat_raw[:, blk, :], rstd_sb[:])
                else:
                    for blk in range(4):
                        nc.vector.tensor_mul(kvl_sb[:, blk, 0:nf],
                                             lat_raw[:, blk, 0:nf], rstd_sb[:, 0:nf])
                    # ship this half: latent + krope -> DRAM -> AllGather
                    t = sup
                    nc.scalar.dma_start(
                        lat_kin[t][0:512, :].rearrange("(c p) n -> p c n", p=128),
                        kvl_sb[:, :, 0:nf])
                    nc.scalar.dma_start(lat_kin[t][512:576, :], kr_sb[:, xs])
                    nc.gpsimd.collective_compute(
                        "AllGather",
                        mybir.AluOpType.bypass,
                        ins=[lat_kin[t][:].rearrange("a b -> (a b)")],
                        outs=[lat_g[t][:].rearrange("w a b -> (w a b)")],
                        replica_groups=[[g * 4 + i for i in range(4)]
                                        for g in range(2)],
                    )

            # load gathered halves (gathered order; attention uses GPERM)
            for t in range(2):
                for c in range(4):
                    nc.gpsimd.dma_start(
                        kvlat_h[t][:, c, :].rearrange("p (w m) -> p w m", w=4),
                        lat_g[t][:, c * 128:(c + 1) * 128, :]
                        .rearrange("w p m -> p w m"))
                nc.gpsimd.dma_start(
                    krope_h[t][0:64, :].rearrange("p (w m) -> p w m", w=4),
                    lat_g[t][:, 512:576, :].rearrange("w p m -> p w m"))
                nc.gpsimd.dma_start(krope_h[t][64:128, :], krope_h[t][0:64, :])

        if DEBUG:
            for t in range(2):
                nc.sync.dma_start(dbg_kvlat[:, :, t * 1024:(t + 1) * 1024],
                                  kvlat_h[t][:])
                nc.sync.dma_start(dbg_krope[:, t * 1024:(t + 1) * 1024],
                                  krope_h[t][:])
            nc.sync.dma_start(dbg_qlat[:], qlat[:])

        ap_cm = tc.tile_pool(name="att_persist", bufs=1)
        ap_pool = ap_cm.__enter__()
        attn_out = ap_pool.tile([128, H, LQ], FP16, name="attn_out")
        mask_t = ap_pool.tile([128, 16, QW], FP16, name="mask_t")
        nc.sync.dma_start(mask_t[:], masks[:])

        # q decompress for all 16 heads up front (only needs qlat; fills the
        # PE while the latent AllGathers are in flight)
        qn_all = ap_pool.tile([128, H, LQ], FP16, name="qn_all")
        qr2_all = ap_pool.tile([128, H // 2, LQ], FP16, name="qr2_all")
        if not fp8:
            with tc.tile_pool(name="qdec_w", bufs=1) as qdec_w, \
                 tc.tile_pool(name="qdec_t", bufs=2) as qdec_t, \
                 tc.tile_pool(name="ps_qd", bufs=3, space="PSUM") as ps_qd:
                wdqn_t = qdec_w.tile([128, 4, H * 128], dtA, name="wdqn_t")
                wdqr2_t = qdec_w.tile([128, 4, H * 128], dtA, name="wdqr2_t")
                nc.sync.dma_start(wdqn_t[:], w_dqn[:])
                nc.sync.dma_start(wdqr2_t[:], w_dqr2[:])
                for h in range(H):
                    ps = ps_qd.tile([128, 512], FP32, name="ps_qn", tag="ps_q")
                    contract(ps[:], wdqn_t, 0, slice(h * 128, (h + 1) * 128),
                             qlat, 0, slice(0, LQ), 4)
                    cp(qn_all[:, h, :], ps[:])
                for pr in range(H // 2):
                    psp = ps_qd.tile([128, 512], FP32, name="ps_p2", tag="ps_q")
                    psr = ps_qd.tile([128, 512], FP32, name="ps_r2", tag="ps_q")
                    contract(psp[:], wdqr2_t, 0,
                             slice(pr * 256, pr * 256 + 128),
                             qlat, 0, slice(0, LQ), 4)
                    contract(psr[:], wdqr2_t, 0,
                             slice(pr * 256 + 128, pr * 256 + 256),
                             qlat, 0, slice(0, LQ), 4)
                    u2 = qdec_t.tile([128, 512], FP16, name="qu2", tag="qu2")
                    v2 = qdec_t.tile([128, 512], FP16, name="qv2", tag="qv2")
                    nc.vector.tensor_mul(u2[:], psp[:], cs_q2_t[:])
                    nc.vector.tensor_mul(v2[:], psr[:], msc_q2_t[:])
                    nc.vector.tensor_add(qr2_all[:, pr, :], u2[:], v2[:])

        # ---------------- Phase B: per head-pass decompress + attention ------
        for hp in range(NPASS):
            hbase = hp * HPP
            with tc.tile_pool(name="dec_w", bufs=1) as dec_w, \
                 tc.tile_pool(name="kv_sb", bufs=1) as kv_sb, \
                 tc.tile_pool(name="q_sb", bufs=1) as q_sb:
                dec_cm = tc.tile_pool(name="dec_t", bufs=2)
                dec_t = dec_cm.__enter__()
                psd_cm = tc.tile_pool(name="ps_dec", bufs=2, space="PSUM")
                ps_dec = psd_cm.__enter__()
                hsl = slice(hbase * 128, (hbase + HPP) * 128)
                wdkn_t = dec_w.tile([128, 4, HPP * 128], dtA, name="wdkn_t",
                                    tag="wdkn_t")
                wdv_t = dec_w.tile([128, 4, HPP * 128], dtA, name="wdv_t",
                                   tag="wdv_t")
                nc.sync.dma_start(wdkn_t[:], w_dkn[:, :, hsl])
                nc.sync.dma_start(wdv_t[:], w_dv[:, :, hsl])
                if fp8:
                    raise NotImplementedError("fp8 path with hoisted qdec")

                # --- kv decompress (8 heads, all 2048 keys) ---
                # v dtype must match pTs dtype in the AV matmul (no mixed
                # fp8 x fp16 matmuls).
                if fp8:
                    knv2 = kv_sb.tile([128, HPP, 2, S], FP8, name="knv2")
                else:
                    k_n = kv_sb.tile([128, HPP, S], FP16, name="k_n")
                v_t = kv_sb.tile([128, NKT, HPP * VH],
                                 FP8 if fp8pt else FP16, name="v_t")
                def kvdec_half(tg):
                    lat = kvlat_h[tg]
                    for ksl in range(2):
                        ks = slice(ksl * 512, (ksl + 1) * 512)
                        ksg = slice((2 * tg + ksl) * 512, (2 * tg + ksl + 1) * 512)
                        for h in range(HPP):
                            ps = ps_dec.tile([128, 512], FP32, name="ps_kn",
                                             tag="ps_d")
                            contract(ps[:], wdkn_t, 0,
                                     slice(h * 128, (h + 1) * 128),
                                     lat, 0, ks, 4)
                            if fp8:
                                cp(knv2[:, h, 0, ksg], ps[:])
                            else:
                                cp(k_n[:, h, ksg], ps[:])
                    for ktl in range(8):
                        kt = 8 * tg + ktl
                        for g in range(2):      # head groups of 4
                            ps = ps_dec.tile([128, 512], FP32, name="ps_v",
                                             tag="ps_d")
                            for i in range(4):
                                nc.tensor.matmul(
                                    ps[:], lat[:, i, ktl * 128:(ktl + 1) * 128],
                                    wdv_t[:, i, g * 512:(g + 1) * 512],
                                    start=(i == 0), stop=(i == 3))
                            cp(v_t[:, kt, g * 512:(g + 1) * 512], ps[:])

                if DEBUG and hp == 0 and not fp8:
                    nc.sync.dma_start(dbg_kn[:], k_n[:])
                    nc.sync.dma_start(dbg_vt[:], v_t[:])
                    nc.sync.dma_start(dbg_qn[:], qn_t[:])
                    nc.sync.dma_start(dbg_qr2[:], qr2_t[:])
                if DEBUG and hp == 0 and fp8:
                    nc.sync.dma_start(dbg_knv2[:], knv2[:])
                    nc.sync.dma_start(dbg_qdr[:], qdr_t[:])
                    nc.sync.dma_start(dbg_vt8[:], v_t[:])

                kvdec_half(0)
                ps_att_cm = tc.tile_pool(name="ps_att", bufs=2, space="PSUM")
                ps_att = ps_att_cm.__enter__()
                ps_avz_cm = tc.tile_pool(name="ps_avz", bufs=2, space="PSUM")
                ps_avz = ps_avz_cm.__enter__()
                att_cm = tc.tile_pool(name="att_t", bufs=2)
                att_t = att_cm.__enter__()

                # --- attention: 8 slots, slot s has 2s+2 key tiles ---
                # pav accumulation groups at F-offsets of one psum bank must
                # be sequential per head (interleaving breaks psum), so keep
                # the whole slot's exp'd probabilities in SBUF, then run each
                # head's AV as one contiguous accumulation group.
                def emit_av(st):
                    s_, pav_, pz_, pTs_ = st
                    trip_ = 2 * s_ + 2
                    qs_ = slice(s_ * QW, (s_ + 1) * QW)
                    for h in range(HPP):
                        fs = slice(h * QW, (h + 1) * QW)
                        vs = slice(h * VH, (h + 1) * VH)
                        if fp8pt:
                            raise NotImplementedError("fp8pt with GPERM")
                        else:
                            for r in range(trip_):
                                nc.tensor.matmul(
                                    pav_[:, fs], v_t[:, GPERM[r], vs],
                                    pTs_[:, r, fs],
                                    start=(r == 0), stop=(r == trip_ - 1))
                    rz = att_t.tile([128, 512], FP32, name="rz", tag="rz")
                    nc.vector.reciprocal(rz[:], pz_[:])
                    nc.vector.tensor_mul(
                        attn_out[:, hbase:hbase + HPP, qs_],
                        pav_[:].rearrange("p (h q) -> p h q", h=HPP),
                        rz[:].rearrange("p (h q) -> p h q", h=HPP))

                pend = None
                for s in range(NSLOT):
                    if s == 4:
                        kvdec_half(1)
                    trip = 2 * s + 2
                    qs = slice(s * QW, (s + 1) * QW)
                    pav = ps_avz.tile([128, 512], FP32, name="pav", tag="pav")
                    pz = ps_avz.tile([128, 512], FP32, name="pz", tag="pz")
                    pTs = att_t.tile([128, NKT, 512], FP8E5 if fp8pt else FP16,
                                     name="pTs", tag="pTs", bufs=2)
                    for r in range(trip):
                        g = GPERM[r]
                        ks = slice(g * 128, (g + 1) * 128)
                        sc = ps_att.tile([128, 512], FP32, name="sc", tag="sc",
                                         bufs=2)
                        for h in range(HPP):
                            fs = slice(h * QW, (h + 1) * QW)
                            if fp8:
                                nc.tensor.matmul(
                                    sc[:, fs], knv2[:, h, :, ks],
                                    qdr_t[:, 2 * h:2 * h + 2, 0, qs],
                                    start=True, stop=True, perf_mode=DRM)
                            else:
                                hg = hbase + h
                                nc.tensor.matmul(sc[:, fs], k_n[:, h, ks],
                                                 qn_all[:, hg, qs],
                                                 start=True, stop=False)
                                ro = (h % 2) * 64
                                nc.tensor.matmul(
                                    sc[:, fs],
                                    krope_h[g // 8][ro:ro + 64,
                                                    (g % 8) * 128:(g % 8) * 128 + 128],
                                    qr2_all[ro:ro + 64, hg // 2, qs],
                                    start=False, stop=True)
                        if r >= 2 * s:      # masked iters (diag + pad)
                            m = 2 * s + (r - 2 * s)
                            scv = sc[:].rearrange("p (h q) -> p h q", h=HPP)
                            mb = mask_t[:, m, :].unsqueeze(1).broadcast_to(
                                [128, HPP, QW])
                            nc.vector.tensor_add(scv, scv, mb)
                        nc.scalar.activation(pTs[:, r, :], sc[:], AF.Exp,
                                             bias=zero1[:])
                        # pz accumulates the full bank (safe to interleave
                        # with sc groups); deferred one iter so the PE never
                        # waits on the exp it just requested.
                        if not fp8pt and r > 0:
                            nc.tensor.matmul(pz[:], ones_h[:], pTs[:, r - 1, :],
                                             start=(r == 1), stop=False)
                        if r == 1 and pend is not None:
                            emit_av(pend)
                            pend = None
                    nc.tensor.matmul(pz[:], ones_h[:], pTs[:, trip - 1, :],
                                     start=False, stop=True)
                    pend = (s, pav, pz, pTs)
                    if DEBUG and hp == 0 and s == 0 and not fp8:
                        nc.sync.dma_start(dbg_pt[:], pTs[:, 0:2, :])
                if pend is not None:
                    emit_av(pend)
                    pend = None
                att_cm.__exit__(None, None, None)
                ps_avz_cm.__exit__(None, None, None)
                ps_att_cm.__exit__(None, None, None)
                psd_cm.__exit__(None, None, None)
                dec_cm.__exit__(None, None, None)

        if DEBUG:
            nc.sync.dma_start(dbg_ao[:], attn_out[:])

        # ---------------- Phase C: output projection -------------------------
        with tc.tile_pool(name="prj_w", bufs=3) as prj_w, \
             tc.tile_pool(name="prj_t", bufs=3) as prj_t, \
             tc.tile_pool(name="ps_prj", bufs=3, space="PSUM") as ps_prj:
            for ob in range(16):
                wp = prj_w.tile([128, 16, 128], FP16, name="wp", tag="wp")
                nc.sync.dma_start(wp[:], w_proj[:, :, ob * 128:(ob + 1) * 128])
                ps = ps_prj.tile([128, 512], FP32, name="ps_o", tag="ps_o")
                for h in range(16):
                    nc.tensor.matmul(ps[:], wp[:, h, :], attn_out[:, h, :],
                                     start=(h == 0), stop=(h == 15))
                ot = prj_t.tile([128, 512], FP32, name="ot", tag="ot")
                cp(ot[:], ps[:])
                nc.scalar.dma_start(out_c[:, ob, :], ot[:])

        ap_cm.__exit__(None, None, None)
        lat_cm.__exit__(None, None, None)
        dram_cm.__exit__(None, None, None)
        const_cm.__exit__(None, None, None)

    nc.compile()
    return nc


def _qsel(j):
    """Local q token order for core with within-batch index j."""
    idx = []
    for s in range(NSLOT):
        t = 4 * s + 3 - j
        idx.extend(range(t * QW, (t + 1) * QW))
    return np.array(idx)


def _prep_inputs(x, freqs_cis, w_cq, w_qnorm, w_dqn, w_dqr, w_ckv, w_kvnorm,
                 w_dkn, w_dv, w_krope, w_proj):
    import ml_dtypes
    f16 = np.float16
    fA = ml_dtypes.float8_e4m3fn if USE_FP8 else f16
    perm = np.concatenate([np.arange(0, ROPE, 2), np.arange(1, ROPE, 2)])
    pe, po = perm[:HALF], perm[HALF:]

    def chunk_major(a, nch):
        # [K, C] -> [128, nch, C] with K = 128*nch
        return np.ascontiguousarray(
            a.reshape(nch, 128, a.shape[1]).transpose(1, 0, 2))

    # compress weights (lhsT layout [K=D, P=out])
    wcq_l = chunk_major(w_cq.T.astype(fA), 16)                    # [128,16,512]
    wkr = (w_krope / H)                                           # [64, D]
    ckx = np.concatenate([w_ckv, wkr[pe], wkr[pe], wkr[po], wkr[po]], axis=0)
    # krope block rows 512:640: p2 = [even;even], r2 = [odd;odd]
    wckx_l = chunk_major(ckx.T.astype(fA), 16)                    # [128,16,640]

    # decompress weights, norm + scale folded
    dqn = (w_dqn * w_qnorm[None, :] * SCALE)                      # [H*128, QR]
    wdqn_l = chunk_major(dqn.T.astype(fA), 4)                     # [128,4,2048]
    dqr = (w_dqr * w_qnorm[None, :] * SCALE).reshape(H, ROPE, QR)
    dqr2 = np.empty((H // 2, 4, HALF * 2, QR), np.float32)
    for p in range(H // 2):
        h0, h1 = 2 * p, 2 * p + 1
        # rows: [x0;x0] for p2 blocks, [x1;x1] for r2 blocks
        dqr2[p, 0, :HALF] = dqr[h0][pe]; dqr2[p, 0, HALF:] = dqr[h0][pe]
        dqr2[p, 1, :HALF] = dqr[h1][pe]; dqr2[p, 1, HALF:] = dqr[h1][pe]
        dqr2[p, 2, :HALF] = dqr[h0][po]; dqr2[p, 2, HALF:] = dqr[h0][po]
        dqr2[p, 3, :HALF] = dqr[h1][po]; dqr2[p, 3, HALF:] = dqr[h1][po]
    # layout per pair: cols [p2_h0(64) p2_h1(64) r2_h0(64) r2_h1(64)]
    dqr2 = dqr2.reshape(H // 2 * 4 * ROPE, QR)                    # [2048, 512]
    wdqr2_l = chunk_major(np.ascontiguousarray(dqr2.T).astype(fA), 4)
    dkn = (w_dkn * w_kvnorm[None, :])
    wdkn_l = chunk_major(dkn.T.astype(fA), 4)
    dvw = (w_dv * w_kvnorm[None, :])
    wdv_l = chunk_major(dvw.T.astype(fA), 4)
    wproj_l = chunk_major(np.ascontiguousarray(w_proj.T).astype(f16), 16)

    cos = freqs_cis[:, :, 0].T.astype(np.float32)                 # [32, S]
    sin = freqs_cis[:, :, 1].T.astype(np.float32)
    cs_kv = np.concatenate([cos, sin], 0).astype(f16)             # [64, S]
    msc_kv = np.concatenate([-sin, cos], 0).astype(f16)

    xT = [np.ascontiguousarray(x[b].T) for b in range(B)]         # [D, S]

    in_maps = []
    for c in range(W):
        b, j = c // 4, c % 4
        qsel = _qsel(j)
        kvsel = np.concatenate([np.arange(128 * (4 * k + j), 128 * (4 * k + j) + 128)
                                for k in range(4)])
        xkv_l = chunk_major(np.ascontiguousarray(xT[b][:, kvsel]).astype(fA), 16)
        xq_l = chunk_major(np.ascontiguousarray(xT[b][:, qsel]).astype(fA), 16)
        csq = cs_kv[:, qsel]
        mscq = msc_kv[:, qsel]
        cs_q2 = np.concatenate([csq, csq], 0)                     # [128, LQ]
        msc_q2 = np.concatenate([mscq, mscq], 0)
        # masks: slot s, d in {0,1} -> iter r = 2s+d, additive 0/-10000
        mk = np.zeros((128, 16, QW), np.float32)
        for s in range(NSLOT):
            for d_ in range(2):
                r = 2 * s + d_
                kg = r * 128 + np.arange(128)
                qg = qsel[s * QW:(s + 1) * QW]
                mk[:, 2 * s + d_, :] = np.where(qg[None, :] >= kg[:, None],
                                                0.0, -10000.0)
        in_maps.append({
            "x_kv": xkv_l, "x_q": xq_l,
            "w_cq": wcq_l, "w_ckx": wckx_l,
            "w_dqn": wdqn_l, "w_dqr2": wdqr2_l, "w_dkn": wdkn_l, "w_dv": wdv_l,
            "w_proj": wproj_l,
            "cs_kv": np.ascontiguousarray(cs_kv[:, kvsel]),
            "msc_kv": np.ascontiguousarray(msc_kv[:, kvsel]),
            "cs_q2": cs_q2.astype(f16), "msc_q2": msc_q2.astype(f16),
            "masks": mk.astype(f16),
        })
    return in_maps


last_results = None


def kernel(x, mask, freqs_cis, w_cq, w_qnorm, w_dqn, w_dqr, w_ckv, w_kvnorm,
           w_dkn, w_dv, w_krope, w_proj):
    global last_results
    if "nc" not in _cache:
        _cache["nc"] = _build()
    nc = _cache["nc"]

    args = [np.asarray(a, np.float32) for a in
            (x, freqs_cis, w_cq, w_qnorm, w_dqn, w_dqr, w_ckv, w_kvnorm,
             w_dkn, w_dv, w_krope, w_proj)]
    in_maps = _prep_inputs(*args)

    res = bass_utils.run_bass_kernel_spmd(nc, in_maps, core_ids=list(range(W)))
    last_results = res

    out = np.empty((B, S, D), np.float32)
    for c in range(W):
        b, j = c // 4, c % 4
        oc = res.results[c]["out_c"]          # [128, 16, 512]
        flat = oc.transpose(1, 0, 2).reshape(D, LQ)
        out[b, _qsel(j), :] = flat.T
    return out
